# revision 9
# baseline (speedup 1.0000x reference)
"""AVWGCN graph-conv kernel for 8 Trainium2 NeuronCores (node-parallel).

out[b,n,o] = sum_ki xg[b,n,ki] * W[n,ki,o] + bias[n,o]
  xg = concat([x, S@x]), S = softmax(relu(E E^T)) row-wise
  W[n] = sum_d E[n,d] Wp[d], bias = E @ bias_pool

Sharding: nodes split 8 ways (625/core, padded to 640). Each core:
  phase 1: A = exp(relu(E E^T)) tiles for its 640 target nodes (bf16, SBUF)
           row-sums r via matmul with ones; recip with zero-pad correction
  phase 2: xg1 = (A^T x) * (1/r)  streamed over all 5120 padded source nodes,
           spilled to DRAM together with xg0 = x (own rows), bf16
  phase 3: per node block: PE-transpose xg to (ki, n, b); per-node weights
           W(ki,n,o) from wp via K=10 matmuls; per-node matmul + K=10 bias
           matmul into PSUM; evict to (b, n, o) and DMA out.
"""

import sys

sys.path.insert(0, "/opt/trn_rl_repo")

from contextlib import ExitStack

import ml_dtypes
import numpy as np

import concourse.bacc as bacc
import concourse.bass as bass
import concourse.mybir as mybir
import concourse.tile as tile
from concourse.masks import make_identity
from concourse.tile import TileContext

F32 = mybir.dt.float32
BF16 = mybir.dt.bfloat16
AF = mybir.ActivationFunctionType
ALU = mybir.AluOpType

FULL_CFG = dict(B=64, C=64, N=5000, GP=640, M=5120, NCORES=8)


def build_nc(cfg):
    B = cfg["B"]  # batches
    C = cfg["C"]  # channels (64)
    GP = cfg["GP"]  # own nodes per core, padded (mult of 128)
    M = cfg["M"]  # total source nodes, padded (mult of 128)
    PADM = M - cfg["N"]  # zero-embedding pad rows among source nodes
    MC = M // 128
    NB = GP // 128
    BG = 8  # batches per bc chunk (512 free)
    BCC = B // BG
    KI = 2 * C  # 128

    nc = bacc.Bacc("TRN2", target_bir_lowering=False)
    xp = nc.dram_tensor("xp", [B, M, C], F32, kind="ExternalInput")
    et = nc.dram_tensor("et", [10, M], F32, kind="ExternalInput")
    eo = nc.dram_tensor("eo", [10, GP], F32, kind="ExternalInput")
    wp = nc.dram_tensor("wp", [10, C, KI], BF16, kind="ExternalInput")
    bp = nc.dram_tensor("bp", [10, C], BF16, kind="ExternalInput")
    out = nc.dram_tensor("out_sh", [B, GP, C], F32, kind="ExternalOutput")

    with TileContext(nc) as tc, ExitStack() as ctx:
        const = ctx.enter_context(tc.tile_pool(name="const", bufs=1))
        ident = const.tile([128, 128], BF16)
        make_identity(nc, ident)
        ones = const.tile([128, 1], BF16)
        nc.vector.memset(ones, 1.0)

        p1e = ExitStack()
        p1c = p1e.enter_context(tc.tile_pool(name="p1c", bufs=1))
        etp = p1c.tile([128, M], F32)
        nc.vector.memset(etp, 0.0)
        nc.sync.dma_start(etp[:10, :], et[:, :])
        eop = const.tile([128, GP], F32)
        nc.vector.memset(eop, 0.0)
        nc.sync.dma_start(eop[:10, :], eo[:, :])
        eob = const.tile([128, GP], BF16)
        nc.vector.tensor_copy(eob, eop)
        wpt = const.tile([128, C, KI], BF16)
        nc.vector.memset(wpt, 0.0)
        nc.sync.dma_start(wpt[:10, :, :], wp[:, :, :])
        bpt = const.tile([128, C], BF16)
        nc.vector.memset(bpt, 0.0)
        nc.sync.dma_start(bpt[:10, :], bp[:, :])

        # ---- phase 1: A tiles (m-part, n-free), bf16, resident in SBUF ----
        abig = const.tile([128, MC, NB, 128], BF16)
        recipr = const.tile([128, NB], F32)
        p1 = ExitStack()
        zps = p1.enter_context(tc.tile_pool(name="zps", bufs=2, space="PSUM"))
        rps = p1.enter_context(tc.tile_pool(name="rps", bufs=2, space="PSUM"))
        p1w = p1.enter_context(tc.tile_pool(name="p1w", bufs=4))
        for nb in range(NB):
            for mc in range(MC):
                zp = zps.tile([128, 128], F32)
                nc.tensor.matmul(
                    zp,
                    lhsT=etp[:, mc * 128 : (mc + 1) * 128],
                    rhs=eop[:, nb * 128 : (nb + 1) * 128],
                    start=True,
                    stop=True,
                )
                ex = p1w.tile([128, 128], F32, tag="ex")
                nc.scalar.activation(ex, zp, AF.Exp)
                # exp(relu(z)) == max(exp(z), 1)
                nc.vector.tensor_scalar(
                    abig[:, mc, nb, :], ex, 1.0, None, ALU.max
                )
        for nb in range(NB):
            rp = rps.tile([128, 1], F32)
            for mc in range(MC):
                nc.tensor.matmul(
                    rp,
                    lhsT=abig[:, mc, nb, :],
                    rhs=ones,
                    start=(mc == 0),
                    stop=(mc == MC - 1),
                )
            rc = p1w.tile([128, 1], F32, tag="rc")
            nc.vector.tensor_scalar_add(rc, rp, -float(PADM))
            nc.vector.reciprocal(recipr[:, nb : nb + 1], rc)
        p1.close()
        p1e.close()

        # ---- phase 2: xg1 = (A^T x) / r, spill xg (both k) to DRAM bf16 ----
        dram = ctx.enter_context(tc.tile_pool(name="dram", bufs=1, space="DRAM"))
        xg_spill = dram.tile([B, GP, 2, C], BF16)
        own0 = cfg["core_mc0"]  # first m-chunk of own nodes (runtime: 5*core)
        p2 = ExitStack()
        xfp = p2.enter_context(tc.tile_pool(name="xfp", bufs=4))
        xbp = p2.enter_context(tc.tile_pool(name="xbp", bufs=MC + 4))
        xg1p = p2.enter_context(tc.tile_pool(name="xg1p", bufs=3))
        xps = p2.enter_context(tc.tile_pool(name="xps", bufs=2, space="PSUM"))
        for bcc in range(BCC):
            bsl = slice(bcc * BG, (bcc + 1) * BG)
            xbt = []
            for mc in range(MC):
                xf = xfp.tile([128, BG, C], F32, tag="xf")
                nc.sync.dma_start(
                    xf, xp[bsl, mc * 128 : (mc + 1) * 128, :].rearrange("b m c -> m b c")
                )
                xb = xbp.tile([128, BG, C], BF16, tag="xb")
                nc.vector.tensor_copy(xb, xf)
                xbt.append(xb)
                moc = mc - own0
                if 0 <= moc < NB:
                    # xg0 (= x at own nodes) straight to spill
                    nc.sync.dma_start(
                        xg_spill[bsl, moc * 128 : (moc + 1) * 128, 0, :].rearrange(
                            "b n c -> n b c"
                        ),
                        xb,
                    )
            for nb in range(NB):
                ps = xps.tile([128, BG, C], F32)
                for mc in range(MC):
                    nc.tensor.matmul(
                        ps,
                        lhsT=abig[:, mc, nb, :],
                        rhs=xbt[mc],
                        start=(mc == 0),
                        stop=(mc == MC - 1),
                    )
                xg1 = xg1p.tile([128, BG, C], BF16)
                nc.vector.tensor_tensor(
                    xg1,
                    ps,
                    recipr[:, nb, None, None].to_broadcast((128, BG, C)),
                    ALU.mult,
                )
                nc.sync.dma_start(
                    xg_spill[bsl, nb * 128 : (nb + 1) * 128, 1, :].rearrange(
                        "b n c -> n b c"
                    ),
                    xg1,
                )

        p2.close()

        # ---- phase 3: transpose, per-node weights, per-node matmul ----
        xgn_p = ctx.enter_context(tc.tile_pool(name="xgn", bufs=1))
        xgt_p = ctx.enter_context(tc.tile_pool(name="xgt", bufs=1))
        wt_p = ctx.enter_context(tc.tile_pool(name="wt", bufs=1))
        osb_p = ctx.enter_context(tc.tile_pool(name="osb", bufs=1))
        tps = ctx.enter_context(tc.tile_pool(name="tps", bufs=2, space="PSUM"))
        wps = ctx.enter_context(tc.tile_pool(name="wps", bufs=2, space="PSUM"))
        ops = ctx.enter_context(tc.tile_pool(name="ops", bufs=2, space="PSUM"))
        for nb in range(NB):
            nsl = slice(nb * 128, (nb + 1) * 128)
            xgn = xgn_p.tile([128, B, KI], BF16)
            nc.sync.dma_start(
                xgn, xg_spill[:, nsl, :, :].rearrange("b n k c -> n b (k c)")
            )
            xgt = xgt_p.tile([128, 128, B], BF16)  # (ki, n, b)
            for b in range(B):
                pt = tps.tile([128, 128], BF16)
                nc.tensor.transpose(pt, xgn[:, b, :], ident)
                nc.any.tensor_copy(xgt[:, :, b], pt)
            wt = wt_p.tile([128, 128, C], BF16)  # (ki, n, o)
            for o in range(C):
                pw = wps.tile([128, 128], F32)
                nc.tensor.matmul(
                    pw, lhsT=wpt[:, o, :], rhs=eob[:, nsl], start=True, stop=True
                )
                nc.any.tensor_copy(wt[:, :, o], pw)
            osb = osb_p.tile([B, 128, C], F32)  # (b, n, o)
            for ng in range(128 // 8):
                po = ops.tile([B, 8, C], F32)
                for j in range(8):
                    nl = ng * 8 + j
                    nc.tensor.matmul(
                        po[:, j, :],
                        lhsT=xgt[:, nl, :],
                        rhs=wt[:, nl, :],
                        start=True,
                        stop=False,
                    )
                    nc.tensor.matmul(
                        po[:, j, :],
                        lhsT=eob[:, nb * 128 + nl, None].to_broadcast((128, B)),
                        rhs=bpt,
                        start=False,
                        stop=True,
                    )
                nc.any.tensor_copy(osb[:, ng * 8 : (ng + 1) * 8, :], po)
            nc.sync.dma_start(out[:, nsl, :], osb)
    nc.compile()
    return nc


_NC_CACHE = {}
TRACE = False
LAST = None


def _get_nc(cfg_key, cfg):
    if cfg_key not in _NC_CACHE:
        _NC_CACHE[cfg_key] = build_nc(cfg)
    return _NC_CACHE[cfg_key]


def prep_inputs(x, node_embedding, weights_pool, bias_pool, cfg, ncores):
    B, C, N, GP, M = cfg["B"], cfg["C"], cfg["N"], cfg["GP"], cfg["M"]
    G = N // ncores  # 625 true nodes per core
    xp = np.zeros((B, M, C), np.float32)
    ep = np.zeros((M, 10), np.float32)
    for g in range(ncores):
        xp[:, g * GP : g * GP + G, :] = x[:, g * G : (g + 1) * G, :]
        ep[g * GP : g * GP + G, :] = node_embedding[g * G : (g + 1) * G, :]
    et = np.ascontiguousarray(ep.T)
    wpt = np.ascontiguousarray(
        weights_pool.transpose(0, 3, 1, 2).reshape(10, C, 2 * C)
    ).astype(ml_dtypes.bfloat16)
    bpt = bias_pool.astype(ml_dtypes.bfloat16)
    in_maps = []
    for c in range(ncores):
        in_maps.append(
            {
                "xp": xp,
                "et": et,
                "eo": np.ascontiguousarray(et[:, c * GP : (c + 1) * GP]),
                "wp": wpt,
                "bp": bpt,
            }
        )
    return in_maps


def kernel(x, node_embedding, weights_pool, bias_pool):
    from concourse.bass_utils import run_bass_kernel_spmd

    x = np.asarray(x, np.float32)
    node_embedding = np.asarray(node_embedding, np.float32)
    weights_pool = np.asarray(weights_pool, np.float32)
    bias_pool = np.asarray(bias_pool, np.float32)

    cfg = dict(FULL_CFG)
    ncores = cfg.pop("NCORES")
    cfg["core_mc0"] = 0  # own nodes always at m-chunks [0, NB) — see below
    B, C, N, GP, M = cfg["B"], cfg["C"], cfg["N"], cfg["GP"], cfg["M"]
    G = N // ncores

    nc = _get_nc(("full", GP, M, B), cfg)

    # SPMD: same program everywhere, so every core's own nodes must sit at
    # the same m-offset. We rotate the node groups per core so that core c's
    # own group is group 0 of ITS xp/et copy. Rotation is just a different
    # group order; A columns only depend on eo (own), A rows follow et order,
    # and xg1 = sum over all m — order-invariant.
    xp0 = np.zeros((B, M, C), np.float32)
    ep0 = np.zeros((M, 10), np.float32)
    for g in range(ncores):
        xp0[:, g * GP : g * GP + G, :] = x[:, g * G : (g + 1) * G, :]
        ep0[g * GP : g * GP + G, :] = node_embedding[g * G : (g + 1) * G, :]
    wpt = np.ascontiguousarray(
        weights_pool.transpose(0, 3, 1, 2).reshape(10, C, 2 * C)
    ).astype(ml_dtypes.bfloat16)
    bpt = bias_pool.astype(ml_dtypes.bfloat16)

    in_maps = []
    for c in range(ncores):
        rot = np.roll(np.arange(ncores), -c)
        xp_c = np.concatenate([xp0[:, g * GP : (g + 1) * GP, :] for g in rot], axis=1)
        ep_c = np.concatenate([ep0[g * GP : (g + 1) * GP, :] for g in rot], axis=0)
        et_c = np.ascontiguousarray(ep_c.T)
        in_maps.append(
            {
                "xp": np.ascontiguousarray(xp_c),
                "et": et_c,
                "eo": np.ascontiguousarray(et_c[:, :GP]),
                "wp": wpt,
                "bp": bpt,
            }
        )

    global LAST
    res = run_bass_kernel_spmd(nc, in_maps, list(range(ncores)), trace=TRACE)
    LAST = res
    outs = res.results
    full = np.concatenate([outs[c]["out_sh"][:, :G, :] for c in range(ncores)], axis=1)
    return full.astype(np.float32)


# revision 10
# speedup vs baseline: 1.7966x; 1.7966x over previous
"""AVWGCN graph-conv kernel for 8 Trainium2 NeuronCores (node-parallel).

out[b,n,o] = sum_ki xg[b,n,ki] * W[n,ki,o] + bias[n,o]
  xg = concat([x, S@x]), S = softmax(relu(E E^T)) row-wise
  W[n] = sum_d E[n,d] Wp[d], bias = E @ bias_pool

Sharding: nodes split 8 ways (625/core, padded to 640). Each core:
  phase 1: A = exp(relu(E E^T)) tiles for its 640 target nodes (bf16, SBUF)
           row-sums r via matmul with ones; recip with zero-pad correction
  phase 2: xg1 = (A^T x) * (1/r)  streamed over all 5120 padded source nodes,
           spilled to DRAM together with xg0 = x (own rows), bf16
  phase 3: per node block: PE-transpose xg to (ki, n, b); per-node weights
           W(ki,n,o) from wp via K=10 matmuls; per-node matmul + K=10 bias
           matmul into PSUM; evict to (b, n, o) and DMA out.
"""

import sys

sys.path.insert(0, "/opt/trn_rl_repo")

from contextlib import ExitStack

import ml_dtypes
import numpy as np

import concourse.bacc as bacc
import concourse.bass as bass
import concourse.mybir as mybir
import concourse.tile as tile
from concourse.masks import make_identity
from concourse.tile import TileContext

F32 = mybir.dt.float32
BF16 = mybir.dt.bfloat16
AF = mybir.ActivationFunctionType
ALU = mybir.AluOpType

FULL_CFG = dict(B=64, C=64, N=5000, GP=640, M=5120, NCORES=8)


def build_nc(cfg):
    B = cfg["B"]  # batches
    C = cfg["C"]  # channels (64)
    GP = cfg["GP"]  # own nodes per core, padded (mult of 128)
    M = cfg["M"]  # total source nodes, padded (mult of 128)
    PADM = M - cfg["N"]  # zero-embedding pad rows among source nodes
    MC = M // 128
    NB = GP // 128
    BG = 8  # batches per bc chunk (512 free)
    BCC = B // BG
    KI = 2 * C  # 128

    nc = bacc.Bacc("TRN2", target_bir_lowering=False)
    xp = nc.dram_tensor("xp", [M, B, C], BF16, kind="ExternalInput")
    et = nc.dram_tensor("et", [10, M], F32, kind="ExternalInput")
    eo = nc.dram_tensor("eo", [10, GP], F32, kind="ExternalInput")
    wp = nc.dram_tensor("wp", [10, C, KI], BF16, kind="ExternalInput")
    bp = nc.dram_tensor("bp", [10, C], BF16, kind="ExternalInput")
    out = nc.dram_tensor("out_sh", [B, GP, C], F32, kind="ExternalOutput")

    with TileContext(nc) as tc, ExitStack() as ctx:
        const = ctx.enter_context(tc.tile_pool(name="const", bufs=1))
        ident = const.tile([128, 128], BF16)
        make_identity(nc, ident)
        ones = const.tile([128, 1], BF16)
        nc.vector.memset(ones, 1.0)

        p1e = ExitStack()
        p1c = p1e.enter_context(tc.tile_pool(name="p1c", bufs=1))
        etp = p1c.tile([128, M], F32)
        nc.vector.memset(etp, 0.0)
        nc.sync.dma_start(etp[:10, :], et[:, :])
        eop = const.tile([128, GP], F32)
        nc.vector.memset(eop, 0.0)
        nc.sync.dma_start(eop[:10, :], eo[:, :])
        eob = const.tile([128, GP], BF16)
        nc.vector.tensor_copy(eob, eop)
        wpt = const.tile([128, C, KI], BF16)
        nc.vector.memset(wpt, 0.0)
        nc.sync.dma_start(wpt[:10, :, :], wp[:, :, :])
        bpt = const.tile([128, C], BF16)
        nc.vector.memset(bpt, 0.0)
        nc.sync.dma_start(bpt[:10, :], bp[:, :])

        # ---- phase 1: A tiles (m-part, n-free), bf16, resident in SBUF ----
        abig = const.tile([128, MC, NB, 128], BF16)
        recipr = const.tile([128, NB], F32)
        p1 = ExitStack()
        zps = p1.enter_context(tc.tile_pool(name="zps", bufs=2, space="PSUM"))
        rps = p1.enter_context(tc.tile_pool(name="rps", bufs=2, space="PSUM"))
        p1w = p1.enter_context(tc.tile_pool(name="p1w", bufs=4))
        for nb in range(NB):
            for mc in range(MC):
                zp = zps.tile([128, 128], F32)
                nc.tensor.matmul(
                    zp,
                    lhsT=etp[:, mc * 128 : (mc + 1) * 128],
                    rhs=eop[:, nb * 128 : (nb + 1) * 128],
                    start=True,
                    stop=True,
                )
                ex = p1w.tile([128, 128], F32, tag="ex")
                nc.scalar.activation(ex, zp, AF.Exp)
                # exp(relu(z)) == max(exp(z), 1)
                nc.vector.tensor_scalar(
                    abig[:, mc, nb, :], ex, 1.0, None, ALU.max
                )
        for nb in range(NB):
            rp = rps.tile([128, 1], F32)
            for mc in range(MC):
                nc.tensor.matmul(
                    rp,
                    lhsT=abig[:, mc, nb, :],
                    rhs=ones,
                    start=(mc == 0),
                    stop=(mc == MC - 1),
                )
            rc = p1w.tile([128, 1], F32, tag="rc")
            nc.vector.tensor_scalar_add(rc, rp, -float(PADM))
            nc.vector.reciprocal(recipr[:, nb : nb + 1], rc)
        p1.close()
        p1e.close()

        # ---- phase 2: xg1 = (A^T x) / r, spill xg (both k) to DRAM bf16 ----
        dram = ctx.enter_context(tc.tile_pool(name="dram", bufs=1, space="DRAM"))
        xg_spill = dram.tile([GP, B, 2, C], BF16)
        own0 = cfg["core_mc0"]  # first m-chunk of own nodes (runtime: 5*core)
        p2 = ExitStack()
        xbp = p2.enter_context(tc.tile_pool(name="xbp", bufs=MC + 4))
        xg1p = p2.enter_context(tc.tile_pool(name="xg1p", bufs=3))
        xps = p2.enter_context(tc.tile_pool(name="xps", bufs=2, space="PSUM"))
        for bcc in range(BCC):
            bsl = slice(bcc * BG, (bcc + 1) * BG)
            xbt = []
            for mc in range(MC):
                xb = xbp.tile([128, BG, C], BF16, tag="xb")
                nc.sync.dma_start(xb, xp[mc * 128 : (mc + 1) * 128, bsl, :])
                xbt.append(xb)
                moc = mc - own0
                if 0 <= moc < NB:
                    # xg0 (= x at own nodes) straight to spill
                    nc.sync.dma_start(
                        xg_spill[moc * 128 : (moc + 1) * 128, bsl, 0, :], xb
                    )
            for nb in range(NB):
                ps = xps.tile([128, BG, C], F32)
                for mc in range(MC):
                    nc.tensor.matmul(
                        ps,
                        lhsT=abig[:, mc, nb, :],
                        rhs=xbt[mc],
                        start=(mc == 0),
                        stop=(mc == MC - 1),
                    )
                xg1 = xg1p.tile([128, BG, C], BF16)
                nc.vector.tensor_tensor(
                    xg1,
                    ps,
                    recipr[:, nb, None, None].to_broadcast((128, BG, C)),
                    ALU.mult,
                )
                nc.sync.dma_start(
                    xg_spill[nb * 128 : (nb + 1) * 128, bsl, 1, :], xg1
                )

        p2.close()

        # ---- phase 3: transpose, per-node weights, per-node matmul ----
        xgn_p = ctx.enter_context(tc.tile_pool(name="xgn", bufs=1))
        xgt_p = ctx.enter_context(tc.tile_pool(name="xgt", bufs=1))
        wt_p = ctx.enter_context(tc.tile_pool(name="wt", bufs=1))
        osb_p = ctx.enter_context(tc.tile_pool(name="osb", bufs=1))
        tps = ctx.enter_context(tc.tile_pool(name="tps", bufs=2, space="PSUM"))
        wps = ctx.enter_context(tc.tile_pool(name="wps", bufs=2, space="PSUM"))
        ops = ctx.enter_context(tc.tile_pool(name="ops", bufs=2, space="PSUM"))
        for nb in range(NB):
            nsl = slice(nb * 128, (nb + 1) * 128)
            xgn = xgn_p.tile([128, B, KI], BF16)
            nc.sync.dma_start(
                xgn, xg_spill[nsl, :, :, :].rearrange("n b k c -> n b (k c)")
            )
            xgt = xgt_p.tile([128, 128, B], BF16)  # (ki, n, b)
            for b in range(B):
                pt = tps.tile([128, 128], BF16)
                nc.tensor.transpose(pt, xgn[:, b, :], ident)
                nc.any.tensor_copy(xgt[:, :, b], pt)
            wt = wt_p.tile([128, 128, C], BF16)  # (ki, n, o)
            for o in range(C):
                pw = wps.tile([128, 128], F32)
                nc.tensor.matmul(
                    pw, lhsT=wpt[:, o, :], rhs=eob[:, nsl], start=True, stop=True
                )
                nc.any.tensor_copy(wt[:, :, o], pw)
            osb = osb_p.tile([B, 128, C], F32)  # (b, n, o)
            for ng in range(128 // 8):
                po = ops.tile([B, 8, C], F32)
                for j in range(8):
                    nl = ng * 8 + j
                    nc.tensor.matmul(
                        po[:, j, :],
                        lhsT=xgt[:, nl, :],
                        rhs=wt[:, nl, :],
                        start=True,
                        stop=False,
                    )
                    nc.tensor.matmul(
                        po[:, j, :],
                        lhsT=eob[:, nb * 128 + nl, None].to_broadcast((128, B)),
                        rhs=bpt,
                        start=False,
                        stop=True,
                    )
                nc.any.tensor_copy(osb[:, ng * 8 : (ng + 1) * 8, :], po)
            nc.sync.dma_start(out[:, nsl, :], osb)
    nc.compile()
    return nc


_NC_CACHE = {}
TRACE = False
LAST = None


def _get_nc(cfg_key, cfg):
    if cfg_key not in _NC_CACHE:
        _NC_CACHE[cfg_key] = build_nc(cfg)
    return _NC_CACHE[cfg_key]


def prep_inputs(x, node_embedding, weights_pool, bias_pool, cfg, ncores):
    B, C, N, GP, M = cfg["B"], cfg["C"], cfg["N"], cfg["GP"], cfg["M"]
    G = N // ncores  # 625 true nodes per core
    xp = np.zeros((B, M, C), np.float32)
    ep = np.zeros((M, 10), np.float32)
    for g in range(ncores):
        xp[:, g * GP : g * GP + G, :] = x[:, g * G : (g + 1) * G, :]
        ep[g * GP : g * GP + G, :] = node_embedding[g * G : (g + 1) * G, :]
    et = np.ascontiguousarray(ep.T)
    wpt = np.ascontiguousarray(
        weights_pool.transpose(0, 3, 1, 2).reshape(10, C, 2 * C)
    ).astype(ml_dtypes.bfloat16)
    bpt = bias_pool.astype(ml_dtypes.bfloat16)
    in_maps = []
    for c in range(ncores):
        in_maps.append(
            {
                "xp": xp,
                "et": et,
                "eo": np.ascontiguousarray(et[:, c * GP : (c + 1) * GP]),
                "wp": wpt,
                "bp": bpt,
            }
        )
    return in_maps


def kernel(x, node_embedding, weights_pool, bias_pool):
    from concourse.bass_utils import run_bass_kernel_spmd

    x = np.asarray(x, np.float32)
    node_embedding = np.asarray(node_embedding, np.float32)
    weights_pool = np.asarray(weights_pool, np.float32)
    bias_pool = np.asarray(bias_pool, np.float32)

    cfg = dict(FULL_CFG)
    ncores = cfg.pop("NCORES")
    cfg["core_mc0"] = 0  # own nodes always at m-chunks [0, NB) — see below
    B, C, N, GP, M = cfg["B"], cfg["C"], cfg["N"], cfg["GP"], cfg["M"]
    G = N // ncores

    nc = _get_nc(("full", GP, M, B), cfg)

    # SPMD: same program everywhere, so every core's own nodes must sit at
    # the same m-offset. We rotate the node groups per core so that core c's
    # own group is group 0 of ITS xp/et copy. Rotation is just a different
    # group order; A columns only depend on eo (own), A rows follow et order,
    # and xg1 = sum over all m — order-invariant.
    x_t = np.ascontiguousarray(x.transpose(1, 0, 2))  # (N, B, C)
    xp0 = np.zeros((M, B, C), ml_dtypes.bfloat16)
    ep0 = np.zeros((M, 10), np.float32)
    for g in range(ncores):
        xp0[g * GP : g * GP + G] = x_t[g * G : (g + 1) * G]
        ep0[g * GP : g * GP + G, :] = node_embedding[g * G : (g + 1) * G, :]
    wpt = np.ascontiguousarray(
        weights_pool.transpose(0, 3, 1, 2).reshape(10, C, 2 * C)
    ).astype(ml_dtypes.bfloat16)
    bpt = bias_pool.astype(ml_dtypes.bfloat16)

    in_maps = []
    for c in range(ncores):
        rot = np.roll(np.arange(ncores), -c)
        xp_c = np.concatenate([xp0[g * GP : (g + 1) * GP] for g in rot], axis=0)
        ep_c = np.concatenate([ep0[g * GP : (g + 1) * GP, :] for g in rot], axis=0)
        et_c = np.ascontiguousarray(ep_c.T)
        in_maps.append(
            {
                "xp": np.ascontiguousarray(xp_c),
                "et": et_c,
                "eo": np.ascontiguousarray(et_c[:, :GP]),
                "wp": wpt,
                "bp": bpt,
            }
        )

    global LAST
    res = run_bass_kernel_spmd(nc, in_maps, list(range(ncores)), trace=TRACE)
    LAST = res
    outs = res.results
    full = np.concatenate([outs[c]["out_sh"][:, :G, :] for c in range(ncores)], axis=1)
    return full.astype(np.float32)
